# Initial kernel scaffold
#
"""Trainium2 Bass kernel for nn_DEC_21732534517794.

Two independent GRU+attention decoders over B=128, L=100, H=128.
Sharding: 8 cores = 2 decoders x 4 batch-shards (B_local=32 per core).
Each core runs one decoder's full 100-step scan for its batch shard in a
transposed [H=128 partitions, B=32 free] layout:

  - GRU gates via PE matmuls into PSUM, sigmoid folded as 0.5+0.5*tanh(0.5x)
    (keeps ACT on one table set: tanh+exp), combine emits h'' = 2*h0n with
    the 0.5 folded into Wih1/WaS/WaK/WfcC/WfcH/Wout host-side.
  - attention caches: kW[h, b*TAL+t] (bf16), keysT[t, b*128+h] (bf16, written
    via PE transpose + DMA reshape).  e = tanh(kW + sW_bcast) computed with a
    rep-8 broadcast trick so the DVE add runs in 2x bf16 mode; scores = Wv.T@e
    on PE (bf16), reshaped to [t-partition, 64] via PSUM->SBUF DMA, exp on ACT,
    weighted sum as 32 tiny PE matmuls (lhsT = E[:, 2b:2b+2]), normalization
    via reciprocal + per-partition tensor_scalar.
  - output: p = (0.5*Wout_half) @ h1n'' collected per step (bf16), projected at
    the end; host adds the two decoder halves (with the D=1 shift), bout, and
    the final sigmoid.
"""

from contextlib import ExitStack

import numpy as np
import ml_dtypes

import concourse.bass as bass
import concourse.bacc as bacc
import concourse.mybir as mybir
import concourse.tile as tile
from concourse.bass_utils import run_bass_kernel_spmd
from concourse.dve_ops import AFFINE_MUL_REDUCE

F32 = mybir.dt.float32
BF16 = mybir.dt.bfloat16
AF = mybir.ActivationFunctionType
H = 128
BC = 32          # batch per core
TAL = 104        # per-b t-columns allocated in kW / e caches
NSTEPS = 100
USE_REP8 = True  # rep-8 sW broadcast => bf16 2x DVE add


def _ap(t, offset, dims):
    """AP on tile t with explicit free dims (partition dim kept full)."""
    base = t[:]
    return bass.AP(tensor=base.tensor, offset=base.offset + offset,
                   ap=[base.ap[0]] + [list(d) for d in dims])


def build(nsteps=NSTEPS):
    nc = bacc.Bacc(trn_type="TRN2")
    L = nsteps

    din = {}
    def dt_in(name, shape, dtype=F32):
        din[name] = nc.dram_tensor(name, shape, dtype, kind="ExternalInput")
        return din[name]

    xT = dt_in("xT", [3, L * BC])
    Wih0T = dt_in("Wih0T", [3, 384])
    Whh0T = dt_in("Whh0T", [128, 384])
    Wih1T = dt_in("Wih1T", [128, 384])
    Whh1T = dt_in("Whh1T", [128, 384])
    brz = dt_in("brz", [128, 4])          # cols: r0,z0,r1,z1
    bin01 = dt_in("bin01", [128, 2])
    bhn_rows = dt_in("bhn_rows", [1, 256])
    ones_row128 = dt_in("ones_row128", [1, 128])
    WaST = dt_in("WaST", [128, 128], BF16)
    WaKT = dt_in("WaKT", [128, 128], BF16)
    Wv2 = dt_in("Wv2", [128, 1], BF16)
    WfcCT = dt_in("WfcCT", [128, 128])
    WfcHT = dt_in("WfcHT", [128, 128])
    bfc1 = dt_in("bfc1", [128, 1])
    WoutX = dt_in("WoutX", [128, 1], BF16)
    I128f = dt_in("I128f", [128, 128])
    I128b = dt_in("I128b", [128, 128], BF16)
    ones_c = dt_in("ones_c", [128, 1], BF16)
    ones_r = dt_in("ones_r", [1, BC])
    mskv = dt_in("mskv", [128, 1])   # -1e30 * sign(Wv)

    p_out = nc.dram_tensor("p_out", [1, L * BC], F32, kind="ExternalOutput")

    with tile.TileContext(nc) as tc, ExitStack() as ctx:
        consts = ctx.enter_context(tc.tile_pool(name="consts", bufs=1))
        state = ctx.enter_context(tc.tile_pool(name="state", bufs=1))
        sp2 = ctx.enter_context(tc.tile_pool(name="sp2", bufs=2))
        sp3 = ctx.enter_context(tc.tile_pool(name="sp3", bufs=3))
        pg = ctx.enter_context(tc.tile_pool(name="pg", bufs=2, space="PSUM"))
        pat = ctx.enter_context(tc.tile_pool(name="pat", bufs=1, space="PSUM"))
        psc = ctx.enter_context(tc.tile_pool(name="psc", bufs=2, space="PSUM"))
        plate = ctx.enter_context(tc.tile_pool(name="plate", bufs=3, space="PSUM"))

        # ---- load constants ----
        cs = {}
        for name, t in din.items():
            sh = list(t.shape)
            tl = consts.tile(sh, t.dtype, tag=name)
            nc.gpsimd.dma_start(out=tl[:, :], in_=t[:, :])
            cs[name] = tl

        # ---- persistent state ----
        # hh: h0 at cols 0:32, h1 at 32:64 (true scale).
        # hhn: interleaved col b*2+q (q=0: 2*h0n, q=1: 2*h1n).
        # kW2: col t*64 + b*2 + a (a in {0,1} duplicated kW values).
        # e_in/e_bf: col t*64 + b*2 + a.
        # keysT2: row j, col par*4096 + b*128 + h = keys[t = 2j+par].
        hh = state.tile([128, 64], F32, tag="hh")
        kW2 = state.tile([128, 64 * TAL], BF16, tag="kW2")
        keysT2 = state.tile([128, 2 * BC * 128], BF16, tag="keysT2")
        e_in = state.tile([128, 64 * TAL], BF16, tag="e_in")
        e_bf = state.tile([128, 64 * TAL], BF16, tag="e_bf")
        o_all = state.tile([128, L * BC], BF16, tag="o_all")
        nc.vector.memset(hh[:, :], 0.0)
        nc.vector.memset(kW2[:, :], 0.0)
        nc.vector.memset(keysT2[:, :], 0.0)
        nc.vector.memset(e_in[:, :], 0.0)
        nc.vector.memset(e_bf[:, :], 0.0)

        MM = nc.tensor.matmul
        ACTV = nc.scalar.activation

        def affine_mul(out, in0, in1, s0, s1):
            nc.vector._custom_dve(AFFINE_MUL_REDUCE, out=out, in0=in0, in1=in1,
                                  s0=float(s0), s1=float(s1))

        def gru_layer(lyr, xg_lhsT, xg_rhs, h_ap, out_ap):
            """One GRU layer. xg_lhsT: [K,384] lhsT for input path, xg_rhs its rhs.
            h_ap: [128,32] state AP. out_ap: [128,32] dest for h''=2*h_new."""
            WhhT = cs["Whh0T"] if lyr == 0 else cs["Whh1T"]
            rcol, zcol = (0, 1) if lyr == 0 else (2, 3)
            prz = pg.tile([128, 64], F32, tag="g")
            pnn = pg.tile([128, 64], F32, tag="g")   # hn | gn
            MM(prz[:, 0:32], xg_lhsT[:, 0:128], xg_rhs, start=True, stop=False)
            MM(prz[:, 0:32], WhhT[:, 0:128], h_ap, start=False, stop=True)
            MM(prz[:, 32:64], xg_lhsT[:, 128:256], xg_rhs, start=True, stop=False)
            MM(prz[:, 32:64], WhhT[:, 128:256], h_ap, start=False, stop=True)
            MM(pnn[:, 0:32], WhhT[:, 256:384], h_ap, start=True, stop=False)
            MM(pnn[:, 0:32], cs["bhn_rows"][0:1, lyr * 128:(lyr + 1) * 128],
               cs["ones_r"][0:1, :], start=False, stop=True)
            MM(pnn[:, 32:64], xg_lhsT[:, 256:384], xg_rhs, start=True, stop=False)
            tr = sp3.tile([128, 32], F32, tag="tr")
            tz = sp3.tile([128, 32], F32, tag="tz")
            ACTV(tr[:, :], prz[:, 0:32], AF.Tanh,
                 bias=cs["brz"][:, rcol:rcol + 1], scale=0.5)
            ACTV(tz[:, :], prz[:, 32:64], AF.Tanh,
                 bias=cs["brz"][:, zcol:zcol + 1], scale=0.5)
            rhn = sp3.tile([128, 32], F32, tag="rhn")
            affine_mul(rhn[:, :], tr[:, :], pnn[:, 0:32], 0.5, 0.5)
            MM(pnn[:, 32:64], cs["I128f"][:, :], rhn[:, :], start=False, stop=True)
            n = sp3.tile([128, 32], F32, tag="n")
            ACTV(n[:, :], pnn[:, 32:64], AF.Tanh,
                 bias=cs["bin01"][:, lyr:lyr + 1], scale=1.0)
            v1 = sp3.tile([128, 32], F32, tag="v1")
            v2 = sp3.tile([128, 32], F32, tag="v2")
            affine_mul(v1[:, :], tz[:, :], h_ap, 1.0, 1.0)       # (1+tz)*h
            affine_mul(v2[:, :], tz[:, :], n[:, :], -1.0, 1.0)   # (1-tz)*n
            nc.vector.tensor_add(out_ap, v1[:, :], v2[:, :])

        for i in range(nsteps):
            T = i + 2
            tpad = min(((T + 7) // 8) * 8, TAL)
            ng = tpad // 8

            hhn = sp2.tile([128, 64], F32, tag="hhn")
            gru_layer(0, cs["Wih0T"][0:3, :], cs["xT"][0:3, i * BC:(i + 1) * BC],
                      hh[:, 0:32], _ap(hhn, 0, [[2, 32]]))
            gru_layer(1, cs["Wih1T"][:, :], _ap(hhn, 0, [[2, 32]]),
                      hh[:, 32:64], _ap(hhn, 1, [[2, 32]]))

            hhn_bf = sp2.tile([128, 64], BF16, tag="hhn_bf")
            nc.vector.tensor_copy(hhn_bf[:, :], hhn[:, :])
            nc.vector.tensor_copy(o_all[:, i * BC:(i + 1) * BC],
                                  _ap(hhn_bf, 1, [[2, 32]]))

            # keysT2 append: transpose [128,64] -> psum [64,128] (rows: b*2+q)
            # -> SBUF -> 2 DMAs into parity-split cache rows t=i (h0n), t=i+1 (h1n)
            ptr = plate.tile([64, 128], BF16, tag="late")
            nc.tensor.transpose(ptr[:, :], hhn_bf[:, :], cs["I128b"][:, :])
            tr_sb = sp2.tile([64, 128], BF16, tag="tr_sb")
            nc.vector.tensor_copy(tr_sb[:, :], ptr[:, :])
            base = tr_sb[:, :]
            for q, t in ((0, i), (1, i + 1)):
                src = bass.AP(tensor=base.tensor, offset=base.offset + q * 128,
                              ap=[[256, 32], [1, 128]])
                nc.gpsimd.dma_start(
                    out=keysT2[t // 2:t // 2 + 1,
                               (t % 2) * 4096:(t % 2) * 4096 + 4096].rearrange(
                        "j (b h) -> j b h", h=128),
                    in_=src)

            # sW | kW matmuls; rhs = hhn_bf (already (b,q)-interleaved)
            patt = pat.tile([128, 128], F32, tag="patt")
            MM(patt[:, 0:64], cs["WaST"][:, :], hhn_bf[:, :], start=True, stop=True)
            MM(patt[:, 64:128], cs["WaKT"][:, :], hhn_bf[:, :], start=True, stop=True)
            sW_bf = sp2.tile([128, 64], BF16, tag="sW_bf")
            nc.vector.tensor_copy(sW_bf[:, :], patt[:, 0:64])
            # kW2 append: h0n-kW (psum cols 2b) -> t=i both a; h1n-kW -> t=i+1
            for q, t in ((0, i), (1, i + 1)):
                nc.vector.tensor_copy(
                    _ap(kW2, t * 64, [[2, 32], [1, 2]]),
                    _ap(patt, 64 + q, [[2, 32], [0, 2]]))

            # e_in = kW + sW (broadcast over t), one DVE op (2x bf16)
            if USE_REP8:
                rep8 = sp2.tile([128, 512], BF16, tag="rep8")
                nc.gpsimd.tensor_copy(
                    out=_ap(rep8, 0, [[64, 8], [1, 64]]),
                    in_=_ap(sW_bf, 0, [[0, 8], [1, 64]]))
                nc.vector.tensor_add(
                    _ap(e_in, 0, [[512, ng], [64, 8], [1, 64]]),
                    _ap(kW2, 0, [[512, ng], [64, 8], [1, 64]]),
                    _ap(rep8, 0, [[0, ng], [64, 8], [1, 64]]))
            else:
                nc.vector.tensor_add(
                    _ap(e_in, 0, [[64, tpad], [1, 64]]),
                    _ap(kW2, 0, [[64, tpad], [1, 64]]),
                    _ap(sW_bf, 0, [[0, tpad], [1, 64]]))

            # e = tanh(e_in) over exact [0, T)
            ACTV(e_bf[:, 0:T * 64], e_in[:, 0:T * 64], AF.Tanh)
            # mask via e: c11 (a=0) excludes t=i+1; at i=0 c12 excludes t=0.
            # e[:, masked] = -1e30*sign(Wv) => score = -inf => exp = 0.
            nc.vector.tensor_scalar(
                _ap(e_bf, (i + 1) * 64, [[2, 32]]),
                _ap(e_bf, (i + 1) * 64, [[2, 32]]),
                0.0, cs["mskv"][:, 0:1],
                mybir.AluOpType.mult, mybir.AluOpType.add)
            if i == 0:
                nc.vector.tensor_scalar(
                    _ap(e_bf, 1, [[2, 32]]),
                    _ap(e_bf, 1, [[2, 32]]),
                    0.0, cs["mskv"][:, 0:1],
                    mybir.AluOpType.mult, mybir.AluOpType.add)

            # scores: lhsT = e-cols for a t-pair -> out partition p = tlo*64+ba
            nj = (T + 1) // 2
            je, jo = (T + 1) // 2, T // 2
            pS = psc.tile([128, 64], F32, tag="ps")
            for c in range(nj):
                tn = min(2, T - 2 * c)
                MM(pS[0:tn * 64, c:c + 1], e_bf[:, 2 * c * 64:(2 * c + tn) * 64],
                   cs["Wv2"][:, 0:1], start=True, stop=True)
            Ea = sp2.tile([128, 64], BF16, tag="Ea")
            ACTV(Ea[:, 0:nj], pS[:, 0:nj], AF.Exp)
            pET = plate.tile([64, 128], BF16, tag="late")
            nc.tensor.transpose(pET[0:nj, :], Ea[:, 0:nj], cs["I128b"][:, :])
            E2 = sp2.tile([64, 128], BF16, tag="E2")
            nc.vector.tensor_copy(E2[0:nj, :], pET[0:nj, :])

            # weighted sum: per b, accumulate even+odd parity blocks
            pcn = plate.tile([128, 64], F32, tag="late")
            for b in range(BC):
                MM(pcn[:, 2 * b:2 * b + 2],
                   keysT2[0:je, b * 128:(b + 1) * 128],
                   E2[0:je, 2 * b:2 * b + 2], start=True, stop=False)
                MM(pcn[:, 2 * b:2 * b + 2],
                   keysT2[0:jo, 4096 + b * 128:4096 + (b + 1) * 128],
                   E2[0:jo, 64 + 2 * b:64 + 2 * b + 2], start=False, stop=True)
            pd = plate.tile([1, 64], F32, tag="late")
            MM(pd[0:1, :], cs["ones_c"][0:je, 0:1], E2[0:je, 0:64],
               start=True, stop=False)
            MM(pd[0:1, :], cs["ones_c"][0:jo, 0:1], E2[0:jo, 64:128],
               start=False, stop=True)
            rd = sp2.tile([1, 64], F32, tag="rd")
            nc.vector.reciprocal_approx_fast(out=rd[0:1, :], in_=pd[0:1, :])
            prdb = plate.tile([128, 64], F32, tag="late")
            MM(prdb[:, :], cs["ones_row128"][0:1, :], rd[0:1, :],
               start=True, stop=True)
            rdb = sp2.tile([128, 64], F32, tag="rdb")
            nc.vector.tensor_copy(rdb[:, :], prdb[:, :])
            cT = sp2.tile([128, 64], F32, tag="cT")
            nc.vector.tensor_mul(cT[:, :], pcn[:, :], rdb[:, :])

            # fc + state update
            pfc = plate.tile([128, 64], F32, tag="late")
            MM(pfc[:, :], cs["WfcCT"][:, :], cT[:, :], start=True, stop=False)
            MM(pfc[:, :], cs["WfcHT"][:, :], hhn[:, :], start=False, stop=True)
            ACTV(_ap(hh, 0, [[32, 2], [1, 32]]), _ap(pfc, 0, [[1, 2], [2, 32]]),
                 AF.Identity, bias=cs["bfc1"][:, 0:1], scale=1.0)

        # ---- output projection: p[col] for col = t*BC+b, 128 cols per matmul
        NP = L * BC
        assert NP % 128 == 0, "nsteps must be a multiple of 4"
        nch = NP // 128
        pp = psc.tile([128, nch], F32, tag="ps")
        for c in range(nch):
            MM(pp[:, c:c + 1], o_all[:, c * 128:(c + 1) * 128],
               cs["WoutX"][:, 0:1], start=True, stop=True)
        p_sb = sp2.tile([128, nch], F32, tag="p_sb")
        nc.vector.tensor_copy(p_sb[:, :], pp[:, :])
        nc.gpsimd.dma_start(
            out=bass.AP(tensor=p_out[:, :].tensor, offset=0,
                        ap=[[0, 1], [1, 128], [128, nch]]),
            in_=p_sb[:, :])

    nc.compile()
    return nc


def make_inmaps(inputs, nsteps=NSTEPS):
    """Host-side sharding + layout. Returns list of 8 in_maps."""
    f32, bf = np.float32, ml_dtypes.bfloat16
    L = nsteps
    r = {k: np.asarray(v, f32) for k, v in inputs.items()}
    Wfc, Wattn, Wout = r["Wfc"], r["Wattn"], r["Wout"]
    I = np.eye(128, dtype=f32)
    common = {
        "WaST": np.ascontiguousarray((0.5 * Wattn[:, :H]).T).astype(bf),
        "WaKT": np.ascontiguousarray((0.5 * Wattn[:, H:]).T).astype(bf),
        "Wv2": r["Wv"].reshape(128, 1).astype(bf),
        "WfcCT": np.ascontiguousarray((0.5 * Wfc[:, :H]).T).astype(f32),
        "WfcHT": np.ascontiguousarray((0.5 * Wfc[:, H:]).T).astype(f32),
        "bfc1": r["bfc"].reshape(128, 1).astype(f32),
        "I128f": I, "I128b": I.astype(bf),
        "ones_c": np.ones((128, 1), bf), "ones_r": np.ones((1, BC), f32),
        "ones_row128": np.ones((1, 128), f32),
        "mskv": (-1e30 * np.sign(r["Wv"])).reshape(128, 1).astype(f32),
    }
    maps = []
    for core in range(8):
        k, shard = core // 4, core % 4
        bsl = slice(shard * BC, (shard + 1) * BC)
        x = r["received"][bsl, :L, :]                       # [32, L, 3]
        xT = np.ascontiguousarray(x.transpose(2, 1, 0)).reshape(3, L * BC)
        brz_cols = np.stack([
            0.5 * (r["bih0"][k][:H] + r["bhh0"][k][:H]),
            0.5 * (r["bih0"][k][H:2 * H] + r["bhh0"][k][H:2 * H]),
            0.5 * (r["bih1"][k][:H] + r["bhh1"][k][:H]),
            0.5 * (r["bih1"][k][H:2 * H] + r["bhh1"][k][H:2 * H])], 1)
        m = dict(common)
        m.update({
            "xT": xT.astype(f32),
            "Wih0T": np.ascontiguousarray(r["Wih0"][k].T).astype(f32),
            "Whh0T": np.ascontiguousarray(r["Whh0"][k].T).astype(f32),
            "Wih1T": np.ascontiguousarray((0.5 * r["Wih1"][k]).T).astype(f32),
            "Whh1T": np.ascontiguousarray(r["Whh1"][k].T).astype(f32),
            "brz": brz_cols.astype(f32),
            "bin01": np.stack([r["bih0"][k][2 * H:], r["bih1"][k][2 * H:]], 1).astype(f32),
            "bhn_rows": np.concatenate(
                [r["bhh0"][k][2 * H:], r["bhh1"][k][2 * H:]]).reshape(1, 256).astype(f32),
            "WoutX": (0.5 * Wout[0, k * H:(k + 1) * H]).reshape(128, 1).astype(bf),
        })
        maps.append(m)
    return maps


_CACHE = {}


def kernel(**inputs) -> np.ndarray:
    nsteps = NSTEPS
    if "nc" not in _CACHE:
        _CACHE["nc"] = build(nsteps)
    nc = _CACHE["nc"]
    maps = make_inmaps(inputs, nsteps)
    res = run_bass_kernel_spmd(nc, maps, core_ids=list(range(8)))
    outs = res.results
    L = nsteps
    B = 128
    p1 = np.zeros((B, L), np.float32)
    p2 = np.zeros((B, L), np.float32)
    for core in range(8):
        k, shard = core // 4, core % 4
        bsl = slice(shard * BC, (shard + 1) * BC)
        p = np.asarray(outs[core]["p_out"]).reshape(L, BC).T   # [32, L]
        (p1 if k == 0 else p2)[bsl] = p
    bout = float(np.asarray(inputs["bout"]).reshape(-1)[0])
    idx = np.minimum(np.arange(L) + 1, L - 1)
    z = p1 + p2[:, idx] + bout
    out = (1.0 / (1.0 + np.exp(-z))).astype(np.float32)[..., None]
    return out



# revision 58
# speedup vs baseline: 1257.0005x; 1257.0005x over previous
"""Trainium2 Bass kernel for nn_DEC_21732534517794.

Two independent GRU+attention decoders over B=128, L=100, H=128.
Sharding: 8 cores = 2 decoders x 4 batch-shards (B_local=32 per core).
Each core runs one decoder's full 100-step scan for its batch shard in a
transposed [H=128 partitions, B=32 free] layout.

Latency-oriented redesign (the per-step serial spine dominates):
  - GRU: all biases folded into PE bias-row matmuls; ONE fused tanh ACT for
    the r|z gate pair ([128,64]); n-gate pre-add on DVE (no PE round-trip);
    all GRU matmuls bf16; state hh and layer outputs held in bf16 directly
    (h'' = 2*h convention, 0.5 folded into downstream weights host-side).
  - attention: e = tanh(kW + sW) with the kW cache appended incrementally;
    the add (DVE, 2x bf16 via stride-0 broadcast of sW) and tanh (ACT) are
    CHUNKED over t so DVE/ACT/PE score-matmuls pipeline instead of
    serializing; scores via t-pair matmuls into [tpair*64+bq, nj] PSUM;
    exp -> PE transpose -> weighted sum as per-b matmuls (LDW is free on
    the PE; engine time is ~2ns per tiny matmul).
  - attention output via a PROJECTED-key cache pkT2 = WfcC @ key (computed
    directly transposed, v-parts as matmul stationary): the weighted sum
    with pre-normalized softmax weights accumulates straight into the fc
    PSUM bank, so the whole normalize/fc tail is exp(+fused row-sum
    accum) -> parity-fold matmul -> reciprocal -> per-partition scale ->
    transpose -> weighted sum -> one state copy.
  - GRU layer outputs stay as (v1, v2) pairs; every consumer matmul
    (layer-1 input path, sW/kW, pk projection, fc H-path) accumulates both
    parts via matmul linearity, keeping the final adds off the spine.
  - cache-append DMAs issue from Pool and SP in parallel; constant loads
    round-robin three engines; o_all add on Pool.
"""

from contextlib import ExitStack

import numpy as np
import ml_dtypes

import concourse.bass as bass
import concourse.bacc as bacc
import concourse.mybir as mybir
import concourse.tile as tile
from concourse.bass_utils import run_bass_kernel_spmd
from concourse.dve_ops import AFFINE_MUL_REDUCE

F32 = mybir.dt.float32
BF16 = mybir.dt.bfloat16
AF = mybir.ActivationFunctionType
H = 128
BC = 32          # batch per core
TAL = 104        # per-b t-columns allocated in kW / e caches
NSTEPS = 100
CHUNK_MIN_T = 24  # single add/tanh chunk below this history length
NCHUNKS = 3


def _ap(t, offset, dims):
    """AP on tile t with explicit free dims (partition dim kept full)."""
    base = t[:]
    return bass.AP(tensor=base.tensor, offset=base.offset + offset,
                   ap=[base.ap[0]] + [list(d) for d in dims])


def build(nsteps=NSTEPS):
    nc = bacc.Bacc(trn_type="TRN2")
    L = nsteps

    din = {}
    def dt_in(name, shape, dtype=BF16):
        din[name] = nc.dram_tensor(name, shape, dtype, kind="ExternalInput")
        return din[name]

    xT = dt_in("xT", [3, L * BC])
    Wih0T = dt_in("Wih0T", [3, 384])
    Whh0T = dt_in("Whh0T", [128, 384])
    Wih1T = dt_in("Wih1T", [128, 384])     # 0.5-scaled
    Whh1T = dt_in("Whh1T", [128, 384])
    brz_rows = dt_in("brz_rows", [1, 512])   # r0|z0|r1|z1 (bih+bhh)
    bhn_rows = dt_in("bhn_rows", [1, 256])   # bhh_n per layer
    bin_rows = dt_in("bin_rows", [1, 256])   # bih_n per layer
    bfc_row = dt_in("bfc_row", [1, 128])
    ones64 = dt_in("ones64", [1, 64])
    WaST = dt_in("WaST", [128, 128])         # 0.5-scaled
    WaKT = dt_in("WaKT", [128, 128])         # 0.5-scaled
    Wv2 = dt_in("Wv2", [128, 1])
    WfcC2 = dt_in("WfcC2", [128, 128])       # (0.5*Wfc[:, :H]).T
    WfcHT = dt_in("WfcHT", [128, 128])       # 0.5-scaled
    WoutX = dt_in("WoutX", [128, 1])         # 0.5-scaled
    I128b = dt_in("I128b", [128, 128])
    # score-mask rows added via PE into pS: -1e30 at masked partitions
    mrowA = dt_in("mrowA", [1, 128])   # q=0 of even-t (partitions 2b)
    mrowB = dt_in("mrowB", [1, 128])   # q=0 of odd-t (partitions 64+2b)
    mrow0 = dt_in("mrow0", [1, 128])   # q=1 of even-t (partitions 2b+1)
    mneg = dt_in("mneg", [1, 64])      # all -1e30: junk-row clear, T odd
    Mdup = dt_in("Mdup", [128, 128], F32)  # M[p',m]=1 iff p'%64==m%64

    p_out = nc.dram_tensor("p_out", [1, L * BC], F32, kind="ExternalOutput")

    with tile.TileContext(nc) as tc, ExitStack() as ctx:
        consts = ctx.enter_context(tc.tile_pool(name="consts", bufs=1))
        state = ctx.enter_context(tc.tile_pool(name="state", bufs=1))
        sp2 = ctx.enter_context(tc.tile_pool(name="sp2", bufs=2))
        sp3 = ctx.enter_context(tc.tile_pool(name="sp3", bufs=3))
        pg = ctx.enter_context(tc.tile_pool(name="pg", bufs=2, space="PSUM"))
        pat = ctx.enter_context(tc.tile_pool(name="pat", bufs=1, space="PSUM"))
        psc = ctx.enter_context(tc.tile_pool(name="psc", bufs=2, space="PSUM"))
        plate = ctx.enter_context(tc.tile_pool(name="plate", bufs=3, space="PSUM"))

        # ---- load constants (round-robin engines so the ~1us per-DMA
        # descriptor generation runs 4-wide instead of serializing) ----
        cs = {}
        dma_engines = (nc.gpsimd, nc.sync, nc.scalar)
        for k, (name, t) in enumerate(din.items()):
            sh = list(t.shape)
            tl = consts.tile(sh, t.dtype, tag=name)
            dma_engines[k % 3].dma_start(out=tl[:, :], in_=t[:, :])
            cs[name] = tl

        # ---- persistent state ----
        # hh: h0 at cols 0:32, h1 at 32:64 (true scale), bf16.
        # GRU layer outputs live as (v1, v2) bf16 pairs (v1+v2 = 2*h_new).
        # kW2: col t*64 + b*2 + a (a in {0,1} duplicated kW values).
        # pkT2: row j, col par*4096 + b*128 + h' = (WfcC @ keys)[t = 2j+par]
        #   -- the fc-PROJECTED key cache: the weighted sum over pkT2 with
        #   normalized softmax weights directly yields the fc C-term.
        hh = state.tile([128, 64], BF16, tag="hh")
        kW2 = state.tile([128, 64 * TAL], BF16, tag="kW2")
        pkT2 = state.tile([128, 2 * BC * 128], BF16, tag="pkT2")
        e_in = state.tile([128, 64 * TAL], BF16, tag="e_in")
        e_bf = state.tile([128, 64 * TAL], BF16, tag="e_bf")
        o_all = state.tile([128, L * BC], BF16, tag="o_all")
        nc.vector.memset(hh[:, :], 0.0)

        MM = nc.tensor.matmul
        ACTV = nc.scalar.activation

        def affine_mul(out, in0, in1, s0, s1):
            nc.vector._custom_dve(AFFINE_MUL_REDUCE, out=out, in0=in0, in1=in1,
                                  s0=float(s0), s1=float(s1))

        def gru_layer(lyr, x_lhsT, x_rhs_list, h_ap):
            """One GRU layer; returns (v1, v2) with v1+v2 = h'' = 2*h_new.

            The layer OUTPUT stays as the (v1, v2) pair: downstream matmuls
            accumulate both parts (matmul linearity), keeping the final add
            off the serial spine.  x_rhs_list carries 1..2 input-path rhs
            operands whose contributions sum (layer 1 feeds v1_0, v2_0)."""
            WhhT = cs["Whh0T"] if lyr == 0 else cs["Whh1T"]
            bo = lyr * 256
            ones = cs["ones64"]
            prz = pg.tile([128, 64], F32, tag="g")
            pnn = pg.tile([128, 64], F32, tag="g")
            # h-path + bias matmuls first: they only need h (prev step), so
            # they run while the x-operand (prior layer) is still in flight.
            MM(prz[:, 0:32], WhhT[:, 0:128], h_ap, start=True, stop=False)
            MM(prz[:, 0:32], cs["brz_rows"][0:1, bo:bo + 128],
               ones[0:1, 0:32], start=False, stop=False)
            MM(prz[:, 32:64], WhhT[:, 128:256], h_ap, start=True, stop=False)
            MM(prz[:, 32:64], cs["brz_rows"][0:1, bo + 128:bo + 256],
               ones[0:1, 0:32], start=False, stop=False)
            MM(pnn[:, 0:32], WhhT[:, 256:384], h_ap, start=True, stop=False)
            MM(pnn[:, 0:32], cs["bhn_rows"][0:1, lyr * 128:(lyr + 1) * 128],
               ones[0:1, 0:32], start=False, stop=True)
            MM(pnn[:, 32:64], cs["bin_rows"][0:1, lyr * 128:(lyr + 1) * 128],
               ones[0:1, 0:32], start=True, stop=False)
            for xi, x_rhs in enumerate(x_rhs_list):
                last = xi == len(x_rhs_list) - 1
                MM(prz[:, 0:32], x_lhsT[:, 0:128], x_rhs,
                   start=False, stop=last)
                MM(prz[:, 32:64], x_lhsT[:, 128:256], x_rhs,
                   start=False, stop=last)
                MM(pnn[:, 32:64], x_lhsT[:, 256:384], x_rhs,
                   start=False, stop=last)
            trz = sp3.tile([128, 64], BF16, tag=f"trz{lyr}")
            ACTV(trz[:, :], prz[:, 0:64], AF.Tanh, scale=0.5)
            rhn = sp3.tile([128, 32], BF16, tag=f"rhn{lyr}")
            affine_mul(rhn[:, :], trz[:, 0:32], pnn[:, 0:32], 0.5, 0.5)
            nin = sp3.tile([128, 32], BF16, tag=f"nin{lyr}")
            nc.vector.tensor_add(nin[:, :], rhn[:, :], pnn[:, 32:64])
            v1 = sp3.tile([128, 32], BF16, tag=f"v1{lyr}")
            affine_mul(v1[:, :], trz[:, 32:64], h_ap, 1.0, 1.0)   # (1+tz)*h
            n = sp3.tile([128, 32], BF16, tag=f"n{lyr}")
            ACTV(n[:, :], nin[:, :], AF.Tanh)
            v2 = sp3.tile([128, 32], BF16, tag=f"v2{lyr}")
            affine_mul(v2[:, :], trz[:, 32:64], n[:, :], -1.0, 1.0)  # (1-tz)*n
            return v1, v2

        for i in range(nsteps):
            T = i + 2
            nj = (T + 1) // 2
            je, jo = nj, T // 2

            v10, v20 = gru_layer(0, cs["Wih0T"][0:3, :],
                                 [cs["xT"][0:3, i * BC:(i + 1) * BC]],
                                 hh[:, 0:32])
            v11, v21 = gru_layer(1, cs["Wih1T"][:, :], [v10[:, :], v20[:, :]],
                                 hh[:, 32:64])
            vq = ((v10, v20), (v11, v21))

            # output collection (off-spine, Pool)
            nc.gpsimd.tensor_add(o_all[:, i * BC:(i + 1) * BC],
                                 v11[:, :], v21[:, :])

            # projected-key cache append, computed directly TRANSPOSED:
            # pkT[q*32+b, h'] = sum_h key[h, (b,q)] * WfcC.T[h, h'] via 4
            # matmuls with the v-tiles as stationary (out rows = q-blocks),
            # then one ACT copy and 2 parallel DMAs (Pool/SP).
            ptr2 = plate.tile([64, 128], F32, tag="late")
            def append_pk_mms():
                for q, (va, vb) in enumerate(vq):
                    MM(ptr2[q * 32:(q + 1) * 32, :], va[:, :],
                       cs["WfcC2"][:, :], start=True, stop=False)
                    MM(ptr2[q * 32:(q + 1) * 32, :], vb[:, :],
                       cs["WfcC2"][:, :], start=False, stop=True)

            def append_pk_dma():
                tr_sb = sp2.tile([64, 128], BF16, tag="tr_sb")
                ACTV(tr_sb[:, :], ptr2[:, :], AF.Identity)
                base = tr_sb[:, :]
                for (q, t), eng in zip(((0, i), (1, i + 1)),
                                       (nc.gpsimd, nc.sync)):
                    src = bass.AP(tensor=base.tensor,
                                  offset=base.offset + q * 4096,
                                  ap=[[128, 32], [1, 128]])
                    eng.dma_start(
                        out=pkT2[t // 2:t // 2 + 1,
                                 (t % 2) * 4096:(t % 2) * 4096 + 4096].rearrange(
                            "j (b h) -> j b h", h=128),
                        in_=src)

            # sW | kW matmuls; v-parts accumulate into interleaved columns.
            # The q=0 sW half is copied as soon as layer 0's parts land, so
            # only the q=1 half-copy sits on the spine.
            patt = pat.tile([128, 128], F32, tag="patt")
            sW_bf = sp2.tile([128, 64], BF16, tag="sW_bf")
            for q, (va, vb) in enumerate(vq):
                MM(_ap(patt, q, [[2, 32]]), cs["WaST"][:, :], va[:, :],
                   start=True, stop=False)
                MM(_ap(patt, q, [[2, 32]]), cs["WaST"][:, :], vb[:, :],
                   start=False, stop=True)
                MM(_ap(patt, 64 + q, [[2, 32]]), cs["WaKT"][:, :], va[:, :],
                   start=True, stop=False)
                MM(_ap(patt, 64 + q, [[2, 32]]), cs["WaKT"][:, :], vb[:, :],
                   start=False, stop=True)
                nc.vector.tensor_copy(_ap(sW_bf, q, [[2, 32]]),
                                      _ap(patt, q, [[2, 32]]))
            append_pk_mms()
            append_pk_dma()

            # e = tanh(kW + sW) chunked over t; scores per t-pair pipelined.
            # First chunk is small so the ACT tanh chain starts early; the
            # kW2 append (only needed by the final chunk, which covers
            # t=i,i+1) is issued between the first and last adds.
            pS = psc.tile([128, 64], F32, tag="ps")
            if T <= CHUNK_MIN_T:
                # split the fresh t=i,i+1 cols (which wait on this step's
                # kW2 append) off the bulk so the first tanh starts early
                bounds = [0, i & ~1, T]
            else:
                rest = T - 8
                step_t = rest // NCHUNKS
                bounds = [0] + [8 + min((k * step_t) & ~1, rest)
                                for k in range(NCHUNKS)] + [T]
            for ci in range(len(bounds) - 1):
                ta, tb = bounds[ci], bounds[ci + 1]
                if ta >= tb:
                    continue
                if tb == T:
                    # kW2 append: h0n-kW (psum cols 2b) -> t=i both a;
                    # h1n-kW -> t=i+1 -- feeds this final chunk's add.
                    for q, t in ((0, i), (1, i + 1)):
                        nc.vector.tensor_copy(
                            _ap(kW2, t * 64, [[2, 32], [1, 2]]),
                            _ap(patt, 64 + q, [[2, 32], [0, 2]]))
                nt = tb - ta
                nc.vector.tensor_add(
                    _ap(e_in, ta * 64, [[64, nt], [1, 64]]),
                    _ap(kW2, ta * 64, [[64, nt], [1, 64]]),
                    _ap(sW_bf, 0, [[0, nt], [1, 64]]))
                ACTV(_ap(e_bf, ta * 64, [[1, nt * 64]]),
                     _ap(e_in, ta * 64, [[1, nt * 64]]), AF.Tanh)
                for c in range(ta // 2, (tb + 1) // 2):
                    tn = min(2, T - 2 * c)
                    # mask via PE bias rows: c11 (q=0) excludes t=i+1; at
                    # i=0 c12 excludes t=0.  masked score += -1e30.
                    masks = []
                    if 2 * c <= i + 1 < 2 * c + tn:
                        masks.append(cs["mrowB"] if (i + 1) % 2 else cs["mrowA"])
                    if i == 0 and c == 0:
                        masks.append(cs["mrow0"])
                    MM(pS[0:tn * 64, c:c + 1],
                       e_bf[:, 2 * c * 64:(2 * c + tn) * 64],
                       cs["Wv2"][:, 0:1], start=True, stop=not masks)
                    for mi, m in enumerate(masks):
                        MM(pS[0:128, c:c + 1], m[0:1, :],
                           cs["ones64"][0:1, 0:1],
                           start=False, stop=mi == len(masks) - 1)
                    if tn == 1:
                        # T odd: rows 64:128 of the last pair hold stale
                        # psum; force exp -> 0 so the row-sum S stays clean.
                        MM(pS[64:128, c:c + 1], cs["mneg"][0:1, :],
                           cs["ones64"][0:1, 0:1], start=True, stop=True)

            # exp with fused per-partition row-sum (softmax denominator
            # halves); fold parities via the 0/1 Mdup matmul, then scale Ea
            # in place => normalized weights BEFORE the transpose.
            Ea = sp2.tile([128, 64], BF16, tag="Ea")
            S = sp2.tile([128, 1], F32, tag="S")
            ACTV(Ea[:, 0:nj], pS[:, 0:nj], AF.Exp, accum_out=S[:, 0:1])
            pdP = plate.tile([128, 1], F32, tag="late")
            MM(pdP[:, 0:1], cs["Mdup"][:, :], S[:, 0:1], start=True, stop=True)
            recipP = sp2.tile([128, 1], F32, tag="recipP")
            nc.vector.reciprocal_approx_fast(out=recipP[:, 0:1], in_=pdP[:, 0:1])
            nc.vector.tensor_scalar_mul(Ea[:, 0:nj], Ea[:, 0:nj],
                                        recipP[:, 0:1])
            pET = plate.tile([64, 128], BF16, tag="late")
            nc.tensor.transpose(pET[0:nj, :], Ea[:, 0:nj], cs["I128b"][:, :])
            E2 = sp2.tile([64, 128], BF16, tag="E2")
            nc.vector.tensor_copy(E2[0:nj, :], pET[0:nj, :])

            # fc: H-path + bias + the weighted sum over PROJECTED keys all
            # accumulate into one bank; hh state update is a plain copy.
            pfc = plate.tile([128, 64], F32, tag="late")
            MM(pfc[:, :], cs["bfc_row"][0:1, :], cs["ones64"][0:1, :],
               start=True, stop=False, skip_group_check=True)
            for q, (va, vb) in enumerate(vq):
                MM(_ap(pfc, q, [[2, 32]]), cs["WfcHT"][:, :], va[:, :],
                   start=False, stop=False, skip_group_check=True)
                MM(_ap(pfc, q, [[2, 32]]), cs["WfcHT"][:, :], vb[:, :],
                   start=False, stop=False, skip_group_check=True)
            for b in range(BC):
                MM(pfc[:, 2 * b:2 * b + 2],
                   pkT2[0:je, b * 128:(b + 1) * 128],
                   E2[0:je, 2 * b:2 * b + 2], start=False, stop=False,
                   skip_group_check=True)
                MM(pfc[:, 2 * b:2 * b + 2],
                   pkT2[0:jo, 4096 + b * 128:4096 + (b + 1) * 128],
                   E2[0:jo, 64 + 2 * b:64 + 2 * b + 2], start=False,
                   stop=b == BC - 1, skip_group_check=True)
            # split state write: h0 lands first so next step's layer-0
            # h-path matmuls release as early as possible
            nc.vector.tensor_copy(hh[:, 0:32], _ap(pfc, 0, [[2, 32]]))
            nc.vector.tensor_copy(hh[:, 32:64], _ap(pfc, 1, [[2, 32]]))

        # ---- output projection: p[col] for col = t*BC+b, 128 cols per matmul
        NP = L * BC
        assert NP % 128 == 0, "nsteps must be a multiple of 4"
        nch = NP // 128
        pp = psc.tile([128, nch], F32, tag="ps")
        for c in range(nch):
            MM(pp[:, c:c + 1], o_all[:, c * 128:(c + 1) * 128],
               cs["WoutX"][:, 0:1], start=True, stop=True)
        p_sb = sp2.tile([128, nch], F32, tag="p_sb")
        nc.vector.tensor_copy(p_sb[:, :], pp[:, :])
        nc.gpsimd.dma_start(
            out=bass.AP(tensor=p_out[:, :].tensor, offset=0,
                        ap=[[0, 1], [1, 128], [128, nch]]),
            in_=p_sb[:, :])

    nc.compile()
    return nc


def _mask_row(*conds):
    m = np.ones(128, bool)
    for c in conds:
        m &= c
    return np.where(m, -1e30, 0.0).astype(np.float32).reshape(1, 128)


def make_inmaps(inputs, nsteps=NSTEPS):
    """Host-side sharding + layout. Returns list of 8 in_maps."""
    f32, bf = np.float32, ml_dtypes.bfloat16
    L = nsteps
    r = {k: np.asarray(v, f32) for k, v in inputs.items()}
    Wfc, Wattn, Wout = r["Wfc"], r["Wattn"], r["Wout"]
    I = np.eye(128, dtype=f32)
    common = {
        "WaST": np.ascontiguousarray((0.5 * Wattn[:, :H]).T).astype(bf),
        "WaKT": np.ascontiguousarray((0.5 * Wattn[:, H:]).T).astype(bf),
        "Wv2": r["Wv"].reshape(128, 1).astype(bf),
        "WfcC2": np.ascontiguousarray((0.5 * Wfc[:, :H]).T).astype(bf),
        "WfcHT": np.ascontiguousarray((0.5 * Wfc[:, H:]).T).astype(bf),
        "bfc_row": r["bfc"].reshape(1, 128).astype(bf),
        "I128b": I.astype(bf),
        "ones64": np.ones((1, 64), bf),
        "mrowA": _mask_row(np.arange(128) % 2 == 0, np.arange(128) < 64).astype(bf),
        "mrowB": _mask_row(np.arange(128) % 2 == 0, np.arange(128) >= 64).astype(bf),
        "mrow0": _mask_row(np.arange(128) % 2 == 1, np.arange(128) < 64).astype(bf),
        "mneg": np.full((1, 64), -1e30, f32).astype(bf),
        "Mdup": (np.arange(128)[:, None] % 64 == np.arange(128)[None, :] % 64
                 ).astype(f32),
    }
    maps = []
    for core in range(8):
        k, shard = core // 4, core % 4
        bsl = slice(shard * BC, (shard + 1) * BC)
        x = r["received"][bsl, :L, :]                       # [32, L, 3]
        xT = np.ascontiguousarray(x.transpose(2, 1, 0)).reshape(3, L * BC)
        brz_rows = np.concatenate([
            r["bih0"][k][:H] + r["bhh0"][k][:H],
            r["bih0"][k][H:2 * H] + r["bhh0"][k][H:2 * H],
            r["bih1"][k][:H] + r["bhh1"][k][:H],
            r["bih1"][k][H:2 * H] + r["bhh1"][k][H:2 * H]]).reshape(1, 512)
        m = dict(common)
        m.update({
            "xT": xT.astype(bf),
            "Wih0T": np.ascontiguousarray(r["Wih0"][k].T).astype(bf),
            "Whh0T": np.ascontiguousarray(r["Whh0"][k].T).astype(bf),
            "Wih1T": np.ascontiguousarray((0.5 * r["Wih1"][k]).T).astype(bf),
            "Whh1T": np.ascontiguousarray(r["Whh1"][k].T).astype(bf),
            "brz_rows": brz_rows.astype(bf),
            "bin_rows": np.concatenate(
                [r["bih0"][k][2 * H:], r["bih1"][k][2 * H:]]).reshape(1, 256).astype(bf),
            "bhn_rows": np.concatenate(
                [r["bhh0"][k][2 * H:], r["bhh1"][k][2 * H:]]).reshape(1, 256).astype(bf),
            "WoutX": (0.5 * Wout[0, k * H:(k + 1) * H]).reshape(128, 1).astype(bf),
        })
        maps.append(m)
    return maps


_CACHE = {}


def kernel(**inputs) -> np.ndarray:
    nsteps = NSTEPS
    if "nc" not in _CACHE:
        _CACHE["nc"] = build(nsteps)
    nc = _CACHE["nc"]
    maps = make_inmaps(inputs, nsteps)
    res = run_bass_kernel_spmd(nc, maps, core_ids=list(range(8)))
    outs = res.results
    L = nsteps
    B = 128
    p1 = np.zeros((B, L), np.float32)
    p2 = np.zeros((B, L), np.float32)
    for core in range(8):
        k, shard = core // 4, core % 4
        bsl = slice(shard * BC, (shard + 1) * BC)
        p = np.asarray(outs[core]["p_out"]).reshape(L, BC).T   # [32, L]
        (p1 if k == 0 else p2)[bsl] = p
    bout = float(np.asarray(inputs["bout"]).reshape(-1)[0])
    idx = np.minimum(np.arange(L) + 1, L - 1)
    z = p1 + p2[:, idx] + bout
    out = (1.0 / (1.0 + np.exp(-z))).astype(np.float32)[..., None]
    return out
